# revision 4
# baseline (speedup 1.0000x reference)
"""Trainium2 Bass kernel: DarkChannelLoss.

Computes -mean(dark_channel(x)) for x [32,3,512,512] f32, where
dark_channel = reflect-pad(7) -> min over channels -> 15x15 sliding-window
min (windows clipped at bottom/right, i.e. +inf padded by 14).

Sharding: pure data parallel over batch, 4 images per NeuronCore x 8 cores.
Each core computes per-partition partial sums of its dark-channel map; the
host combines them into the final scalar mean.

Per-core pipeline (shapes hardcoded), images processed in pairs (bi=2):
  load:   all 8 SWDGE cast DMAs (f32->bf16) issued up front, one per
          (pair, row-tile): 1.5 MB read each, source rows on partitions.
  pass 1: per row-tile: channel-min on DVE, reflect pads along W (DVE
          reversed copies, 4x mode), sliding-min cascade along W (windows
          2,4,8,15) on DVE; INF pad memsets ride on GpSimd (Pool).
          The last row-tile's final combine is split in two so the first
          W-blocks can transpose early.
  transpose: TensorE blocks (identity matmul) into per-W-tile PSUM banks,
          W-block-major order; the narrow last W-tile uses a rectangular
          128x32 transpose. One ScalarE evacuation per W-tile.
  pass 2: H-cascades folded two W-tiles per op ([128, 2, bi, ~539] APs);
          per-fold sums on ScalarE via activation(Copy, accum_out=...).
          The narrow last W-tile (14 cols) of all 4 images is packed into
          one tile at partition offsets {0,32,64,96} and swept once.
  out:    [128, 5] f32 per-partition partial sums; host reduces.
"""

import numpy as np

try:
    import concourse.bass as bass
except ImportError:  # pragma: no cover
    import sys

    sys.path.insert(0, "/opt/trn_rl_repo")
    import concourse.bass as bass

import concourse.mybir as mybir
import concourse.bacc as bacc
from concourse.tile import TileContext
from concourse.bass_utils import run_bass_kernel_spmd

F32 = mybir.dt.float32
BF16 = mybir.dt.bfloat16
INF = float("inf")
MIN = mybir.AluOpType.min
COPY = mybir.ActivationFunctionType.Copy

B, C, H, W = 32, 3, 512, 512
WIN = 15
PAD = WIN // 2          # 7
HP = H + 2 * PAD        # 526 padded rows
WP = W + 2 * PAD        # 526 padded cols
N_CORES = 8
N_IMG = B // N_CORES    # 4 images per core
NT = H // 128           # 4 row tiles of source rows
PT = (WP + 127) // 128  # 5 W tiles (4 full + 1 narrow 14-col)
MF = 544                # m/cascade/tb tile free width (col = src_w + 8)
DEN = B * HP * WP


def build_program(n_img=N_IMG, bi=2):
    assert n_img % bi == 0
    nb = n_img // bi
    nc = bacc.Bacc("TRN2", target_bir_lowering=False, debug=False)
    x = nc.dram_tensor("x", [n_img, C, H, W], F32, kind="ExternalInput")

    n_acc = nb * 2 + 1
    out = nc.dram_tensor("out", [128, n_acc], F32, kind="ExternalOutput")

    n2w = WP + WIN - 2  # 539
    n4w = n2w - 2       # 537
    n8w = n4w - 4       # 533

    with TileContext(nc) as tc:
        from contextlib import ExitStack

        with ExitStack() as ctx:
            constp = ctx.enter_context(tc.tile_pool(name="const", bufs=1))
            chp = ctx.enter_context(tc.tile_pool(name="ch", bufs=2))
            tmpp = ctx.enter_context(tc.tile_pool(name="tmp", bufs=2))
            mp = ctx.enter_context(tc.tile_pool(name="m", bufs=2))
            cascp = ctx.enter_context(tc.tile_pool(name="casc", bufs=4))
            rmp = ctx.enter_context(tc.tile_pool(name="rm", bufs=3))
            tbp = ctx.enter_context(tc.tile_pool(name="tb", bufs=2))
            tb4p = ctx.enter_context(tc.tile_pool(name="tb4", bufs=2))
            dcp = ctx.enter_context(tc.tile_pool(name="dc", bufs=2))
            dcsp = ctx.enter_context(tc.tile_pool(name="dcs", bufs=2))
            accp = ctx.enter_context(tc.tile_pool(name="acc", bufs=1))
            psp = ctx.enter_context(tc.tile_pool(name="ps", bufs=1, space="PSUM"))

            # ---- loads first: all 8 cast DMAs issued before anything else ----
            ch = {}
            for b in range(nb):
                for t in range(NT):
                    ch[b, t] = chp.tile(
                        [128, bi, C, W], BF16, tag=f"ch{t}", name=f"ch_{b}_{t}"
                    )
            for b in range(nb):
                for t in range(NT):
                    nc.gpsimd.dma_start(
                        ch[b, t][:, :, :, :],
                        x[bi * b : bi * (b + 1), :, 128 * t : 128 * (t + 1), :]
                        .rearrange("b c p w -> p b c w"),
                    )

            # ---- constants (identity for TensorE transpose) ----
            ident = constp.tile([128, 128], BF16, tag="ident")
            idt = constp.tile([128, 128], mybir.dt.int16, tag="idt")
            nc.gpsimd.iota(idt[:, :], pattern=[[1, 128]], base=0, channel_multiplier=-1)
            nc.vector.tensor_single_scalar(
                ident[:, :], idt[:, :], 0, mybir.AluOpType.is_equal
            )
            acc = accp.tile([128, n_acc], F32, tag="acc")
            nc.vector.memset(acc[:, :], 0.0)
            # packed tile for the narrow last W-tile of all n_img images:
            # image i sits at partitions 32i..32i+14; other lanes stay 0.0
            tbP4 = accp.tile([128, MF], BF16, tag="tbP4")
            nc.vector.memset(tbP4[:, :], 0.0)

            for b in range(nb):
                tbm = tbp.tile([128, PT - 1, bi, MF], BF16, tag="tb", name=f"tb_{b}")
                tb4 = tb4p.tile([128, bi, MF], BF16, tag="tb4", name=f"tb4_{b}")
                nc.gpsimd.memset(tbm[:, :, :, WP:MF], INF)
                nc.gpsimd.memset(tb4[0:32, :, WP:MF], INF)
                pst = [
                    psp.tile([128, bi, NT, 128], BF16, tag=f"pst{p}", name=f"pst_{b}_{p}")
                    for p in range(PT)
                ]

                # ---- pass 1 per row-tile ----
                for t in range(NT):
                    tmp = tmpp.tile([128, bi, W], BF16, tag="tmp", name=f"tmp_{b}_{t}")
                    nc.vector.tensor_tensor(
                        tmp[:, :, :], ch[b, t][:, :, 0], ch[b, t][:, :, 1], MIN
                    )
                    m = mp.tile([128, bi, MF], BF16, tag="m", name=f"m_{b}_{t}")
                    nc.gpsimd.memset(m[:, :, 8 + W : MF], INF)
                    nc.vector.tensor_tensor(
                        m[:, :, 8 : 8 + W], tmp[:, :, :], ch[b, t][:, :, 2], MIN
                    )
                    # reflect pads on DVE (reversed copies run in 2x/4x mode):
                    # padded 0..6 <- cols 15..9; 519..525 <- 518..512
                    nc.vector.tensor_copy(m[:, :, 1:8], m[:, :, 15:8:-1])
                    nc.vector.tensor_copy(m[:, :, 520:527], m[:, :, 518:511:-1])

                    w2 = cascp.tile([128, bi, MF], BF16, tag="casc", name=f"w2_{b}_{t}")
                    w4 = cascp.tile([128, bi, MF], BF16, tag="casc", name=f"w4_{b}_{t}")
                    w8 = cascp.tile([128, bi, MF], BF16, tag="casc", name=f"w8_{b}_{t}")
                    nc.vector.tensor_tensor(
                        w2[:, :, 0:n2w], m[:, :, 1 : n2w + 1], m[:, :, 2 : n2w + 2], MIN
                    )
                    nc.vector.tensor_tensor(
                        w4[:, :, 0:n4w], w2[:, :, 0:n4w], w2[:, :, 2 : n4w + 2], MIN
                    )
                    nc.vector.tensor_tensor(
                        w8[:, :, 0:n8w], w4[:, :, 0:n8w], w4[:, :, 4 : n8w + 4], MIN
                    )
                    rm = rmp.tile([128, bi, MF], BF16, tag="rm", name=f"rm_{b}_{t}")
                    nc.gpsimd.memset(rm[:, :, WP:MF], INF)
                    if t == NT - 1:
                        # split so the first W-blocks can transpose early
                        nc.vector.tensor_tensor(
                            rm[:, :, 0:256], w8[:, :, 0:256],
                            w8[:, :, PAD : 256 + PAD], MIN
                        )
                        nc.vector.tensor_tensor(
                            rm[:, :, 256:WP], w8[:, :, 256:WP],
                            w8[:, :, 256 + PAD : WP + PAD], MIN
                        )
                    else:
                        nc.vector.tensor_tensor(
                            rm[:, :, 0:WP], w8[:, :, 0:WP], w8[:, :, PAD : WP + PAD], MIN
                        )
                    # transposes, W-block-major so evacs can start early
                    for p in range(PT - 1):
                        for ii in range(bi):
                            nc.tensor.transpose(
                                pst[p][:, ii, t, :],
                                rm[:, ii, 128 * p : 128 * (p + 1)],
                                ident[:, :],
                            )
                    for ii in range(bi):
                        nc.tensor.transpose(
                            pst[PT - 1][0:32, ii, t, :],
                            rm[:, ii, 512:MF],
                            ident[:, :],
                        )

                # ---- evac + reflect for ALL W-tiles (ACT) ----
                for p in range(PT - 1):
                    nc.scalar.copy(
                        tbm[:, p, :, PAD : PAD + H]
                        .rearrange("a b (t w) -> a b t w", t=NT),
                        pst[p][:, :, :, :],
                    )
                for ph in range(2):
                    tbv = tbm[:, 2 * ph : 2 * ph + 2]
                    nc.scalar.copy(tbv[:, :, :, 0:PAD], tbv[:, :, :, 2 * PAD : PAD : -1])
                    nc.scalar.copy(
                        tbv[:, :, :, H + PAD : HP], tbv[:, :, :, H + PAD - 2 : H - 2 : -1]
                    )
                nc.scalar.copy(
                    tb4[0:32, :, PAD : PAD + H]
                    .rearrange("a b (t w) -> a b t w", t=NT),
                    pst[PT - 1][0:32, :, :, :],
                )
                nc.scalar.copy(tb4[0:32, :, 0:PAD], tb4[0:32, :, 2 * PAD : PAD : -1])
                nc.scalar.copy(
                    tb4[0:32, :, H + PAD : HP], tb4[0:32, :, H + PAD - 2 : H - 2 : -1]
                )
                for ii in range(bi):
                    i = bi * b + ii
                    nc.scalar.copy(tbP4[32 * i : 32 * i + 14, :], tb4[0:14, ii, :])

                # ---- pass 2, two W-tiles per op ----
                for ph in range(2):
                    tbv = tbm[:, 2 * ph : 2 * ph + 2]
                    h2 = cascp.tile([128, 2, bi, MF], BF16, tag="hcasc", name=f"h2_{b}_{ph}")
                    h4 = cascp.tile([128, 2, bi, MF], BF16, tag="hcasc", name=f"h4_{b}_{ph}")
                    h8 = cascp.tile([128, 2, bi, MF], BF16, tag="hcasc", name=f"h8_{b}_{ph}")
                    nc.vector.tensor_tensor(
                        h2[:, :, :, 0:n2w], tbv[:, :, :, 0:n2w], tbv[:, :, :, 1 : n2w + 1], MIN
                    )
                    nc.vector.tensor_tensor(
                        h4[:, :, :, 0:n4w], h2[:, :, :, 0:n4w], h2[:, :, :, 2 : n4w + 2], MIN
                    )
                    nc.vector.tensor_tensor(
                        h8[:, :, :, 0:n8w], h4[:, :, :, 0:n8w], h4[:, :, :, 4 : n8w + 4], MIN
                    )
                    dc = dcp.tile([128, 2, bi, HP], BF16, tag="dc", name=f"dc_{b}_{ph}")
                    nc.vector.tensor_tensor(
                        dc[:, :, :, :], h8[:, :, :, 0:HP], h8[:, :, :, PAD : HP + PAD], MIN
                    )
                    # per-fold sum on ScalarE (ACT accumulator)
                    k = b * 2 + ph
                    dcs = dcsp.tile([128, 2, bi, HP], BF16, tag="dcs", name=f"dcs_{b}_{ph}")
                    nc.scalar.activation(
                        dcs[:, :, :, :], dc[:, :, :, :], COPY,
                        accum_out=acc[:, k : k + 1],
                    )

            # ---- packed last W-tile: one cascade for all images ----
            g2 = cascp.tile([128, MF], BF16, tag="casc", name="g2")
            g4 = cascp.tile([128, MF], BF16, tag="casc", name="g4")
            g8 = cascp.tile([128, MF], BF16, tag="casc", name="g8")
            nc.vector.tensor_tensor(g2[:, 0:n2w], tbP4[:, 0:n2w], tbP4[:, 1 : n2w + 1], MIN)
            nc.vector.tensor_tensor(g4[:, 0:n4w], g2[:, 0:n4w], g2[:, 2 : n4w + 2], MIN)
            nc.vector.tensor_tensor(g8[:, 0:n8w], g4[:, 0:n8w], g4[:, 4 : n8w + 4], MIN)
            gdc = dcp.tile([128, HP], BF16, tag="gdc", name="gdc")
            nc.vector.tensor_tensor(gdc[:, :], g8[:, 0:HP], g8[:, PAD : HP + PAD], MIN)
            gdcs = dcsp.tile([128, HP], BF16, tag="gdcs", name="gdcs")
            nc.scalar.activation(
                gdcs[:, :], gdc[:, :], COPY,
                accum_out=acc[:, n_acc - 1 : n_acc],
            )

            nc.sync.dma_start(out[:, :], acc[:, :])

    return nc


_PROGRAM = None


def _get_program():
    global _PROGRAM
    if _PROGRAM is None:
        _PROGRAM = build_program()
        _PROGRAM.finalize()  # run Bacc passes (wait splitting, regalloc)
    return _PROGRAM


def kernel(generated_image):
    x = np.ascontiguousarray(np.asarray(generated_image), dtype=np.float32)
    assert x.shape == (B, C, H, W)
    nc = _get_program()
    shards = x.reshape(N_CORES, N_IMG, C, H, W)
    in_maps = [{"x": np.ascontiguousarray(shards[i])} for i in range(N_CORES)]
    res = run_bass_kernel_spmd(nc, in_maps, list(range(N_CORES)))
    total = float(np.sum([r["out"].astype(np.float64).sum() for r in res.results]))
    return np.array(-total / DEN, dtype=np.float32)


# revision 5
# speedup vs baseline: 1.0631x; 1.0631x over previous
"""Trainium2 Bass kernel: DarkChannelLoss.

Computes -mean(dark_channel(x)) for x [32,3,512,512] f32, where
dark_channel = reflect-pad(7) -> min over channels -> 15x15 sliding-window
min (windows clipped at bottom/right, i.e. +inf padded by 14).

Sharding: pure data parallel over batch, 4 images per NeuronCore x 8 cores.
Each core computes per-partition partial sums of its dark-channel map; the
host combines them into the final scalar mean.

Per-core pipeline (shapes hardcoded), images processed in pairs (bi=2):
  load:   all 8 SWDGE cast DMAs (f32->bf16) issued up front, one per
          (pair, row-tile): 1.5 MB read each, source rows on partitions.
  pass 1: per row-tile: channel-min on DVE, reflect pads along W (ACT
          reversed copies), sliding-min cascade along W (windows 2,4,8,15)
          on DVE. The last row-tile's final combine is split in two so the
          first W-blocks can transpose early.
  transpose: TensorE blocks (identity matmul) into per-W-tile PSUM banks,
          W-block-major order; the narrow last W-tile uses a rectangular
          128x32 transpose. One ScalarE evacuation per W-tile.
  pass 2: per W-tile: sliding-min cascade along H on DVE; per-tile sums on
          ScalarE via activation(Copy, accum_out=...). The narrow last
          W-tile (14 cols) of all 4 images is packed into one tile at
          partition offsets {0,32,64,96}; its cascade runs during the
          last pair's evacuation window to hide the tail.
  out:    [128, 9] f32 per-partition partial sums; host reduces.
"""

import numpy as np

try:
    import concourse.bass as bass
except ImportError:  # pragma: no cover
    import sys

    sys.path.insert(0, "/opt/trn_rl_repo")
    import concourse.bass as bass

import concourse.mybir as mybir
import concourse.bacc as bacc
from concourse.tile import TileContext
from concourse.bass_utils import run_bass_kernel_spmd

F32 = mybir.dt.float32
BF16 = mybir.dt.bfloat16
INF = float("inf")
MIN = mybir.AluOpType.min
COPY = mybir.ActivationFunctionType.Copy

B, C, H, W = 32, 3, 512, 512
WIN = 15
PAD = WIN // 2          # 7
HP = H + 2 * PAD        # 526 padded rows
WP = W + 2 * PAD        # 526 padded cols
N_CORES = 8
N_IMG = B // N_CORES    # 4 images per core
NT = H // 128           # 4 row tiles of source rows
PT = (WP + 127) // 128  # 5 W tiles (4 full + 1 narrow 14-col)
MF = 544                # m/cascade/tb tile free width (col = src_w + 8)
DEN = B * HP * WP


def build_program(n_img=N_IMG, bi=2):
    assert n_img % bi == 0
    nb = n_img // bi
    nc = bacc.Bacc("TRN2", target_bir_lowering=False, debug=False)
    x = nc.dram_tensor("x", [n_img, C, H, W], F32, kind="ExternalInput")

    n_acc = nb * (PT - 1) + 1
    out = nc.dram_tensor("out", [128, n_acc], F32, kind="ExternalOutput")

    n2w = WP + WIN - 2  # 539
    n4w = n2w - 2       # 537
    n8w = n4w - 4       # 533

    with TileContext(nc) as tc:
        from contextlib import ExitStack

        with ExitStack() as ctx:
            constp = ctx.enter_context(tc.tile_pool(name="const", bufs=1))
            chp = ctx.enter_context(tc.tile_pool(name="ch", bufs=2))
            tmpp = ctx.enter_context(tc.tile_pool(name="tmp", bufs=2))
            mp = ctx.enter_context(tc.tile_pool(name="m", bufs=2))
            cascp = ctx.enter_context(tc.tile_pool(name="casc", bufs=4))
            rmp = ctx.enter_context(tc.tile_pool(name="rm", bufs=3))
            tbp = ctx.enter_context(tc.tile_pool(name="tb", bufs=2))
            tb4p = ctx.enter_context(tc.tile_pool(name="tb4", bufs=2))
            dcp = ctx.enter_context(tc.tile_pool(name="dc", bufs=3))
            dcsp = ctx.enter_context(tc.tile_pool(name="dcs", bufs=2))
            accp = ctx.enter_context(tc.tile_pool(name="acc", bufs=1))
            psp = ctx.enter_context(tc.tile_pool(name="ps", bufs=1, space="PSUM"))

            # ---- loads first: all 8 cast DMAs issued before anything else ----
            ch = {}
            for b in range(nb):
                for t in range(NT):
                    ch[b, t] = chp.tile(
                        [128, bi, C, W], BF16, tag=f"ch{t}", name=f"ch_{b}_{t}"
                    )
            for b in range(nb):
                for t in range(NT):
                    nc.gpsimd.dma_start(
                        ch[b, t][:, :, :, :],
                        x[bi * b : bi * (b + 1), :, 128 * t : 128 * (t + 1), :]
                        .rearrange("b c p w -> p b c w"),
                    )

            # ---- constants (identity for TensorE transpose) ----
            ident = constp.tile([128, 128], BF16, tag="ident")
            idt = constp.tile([128, 128], mybir.dt.int16, tag="idt")
            nc.gpsimd.iota(idt[:, :], pattern=[[1, 128]], base=0, channel_multiplier=-1)
            nc.vector.tensor_single_scalar(
                ident[:, :], idt[:, :], 0, mybir.AluOpType.is_equal
            )
            acc = accp.tile([128, n_acc], F32, tag="acc")
            nc.vector.memset(acc[:, :], 0.0)
            # packed tile for the narrow last W-tile of all n_img images:
            # image i sits at partitions 32i..32i+14; other lanes stay 0.0
            tbP4 = accp.tile([128, MF], BF16, tag="tbP4")
            nc.vector.memset(tbP4[:, :], 0.0)

            for b in range(nb):
                last = b == nb - 1
                tbm = tbp.tile([128, PT - 1, bi, MF], BF16, tag="tb", name=f"tb_{b}")
                tb4 = tb4p.tile([128, bi, MF], BF16, tag="tb4", name=f"tb4_{b}")
                nc.vector.memset(tbm[:, :, :, WP:MF], INF)
                nc.vector.memset(tb4[0:32, :, WP:MF], INF)
                pst = [
                    psp.tile([128, bi, NT, 128], BF16, tag=f"pst{p}", name=f"pst_{b}_{p}")
                    for p in range(PT)
                ]

                # ---- pass 1 per row-tile ----
                for t in range(NT):
                    tmp = tmpp.tile([128, bi, W], BF16, tag="tmp", name=f"tmp_{b}_{t}")
                    nc.vector.tensor_tensor(
                        tmp[:, :, :], ch[b, t][:, :, 0], ch[b, t][:, :, 1], MIN
                    )
                    m = mp.tile([128, bi, MF], BF16, tag="m", name=f"m_{b}_{t}")
                    nc.vector.memset(m[:, :, 8 + W : MF], INF)
                    nc.vector.tensor_tensor(
                        m[:, :, 8 : 8 + W], tmp[:, :, :], ch[b, t][:, :, 2], MIN
                    )
                    # reflect pads: padded 0..6 <- cols 15..9; 519..525 <- 518..512
                    nc.scalar.copy(m[:, :, 1:8], m[:, :, 15:8:-1])
                    nc.scalar.copy(m[:, :, 520:527], m[:, :, 518:511:-1])

                    w2 = cascp.tile([128, bi, MF], BF16, tag="casc", name=f"w2_{b}_{t}")
                    w4 = cascp.tile([128, bi, MF], BF16, tag="casc", name=f"w4_{b}_{t}")
                    w8 = cascp.tile([128, bi, MF], BF16, tag="casc", name=f"w8_{b}_{t}")
                    nc.vector.tensor_tensor(
                        w2[:, :, 0:n2w], m[:, :, 1 : n2w + 1], m[:, :, 2 : n2w + 2], MIN
                    )
                    nc.vector.tensor_tensor(
                        w4[:, :, 0:n4w], w2[:, :, 0:n4w], w2[:, :, 2 : n4w + 2], MIN
                    )
                    nc.vector.tensor_tensor(
                        w8[:, :, 0:n8w], w4[:, :, 0:n8w], w4[:, :, 4 : n8w + 4], MIN
                    )
                    rm = rmp.tile([128, bi, MF], BF16, tag="rm", name=f"rm_{b}_{t}")
                    nc.vector.memset(rm[:, :, WP:MF], INF)
                    if t == NT - 1:
                        # split so the first W-blocks can transpose early
                        nc.vector.tensor_tensor(
                            rm[:, :, 0:256], w8[:, :, 0:256],
                            w8[:, :, PAD : 256 + PAD], MIN
                        )
                        nc.vector.tensor_tensor(
                            rm[:, :, 256:WP], w8[:, :, 256:WP],
                            w8[:, :, 256 + PAD : WP + PAD], MIN
                        )
                    else:
                        nc.vector.tensor_tensor(
                            rm[:, :, 0:WP], w8[:, :, 0:WP], w8[:, :, PAD : WP + PAD], MIN
                        )
                    # transposes, W-block-major so evacs can start early
                    for p in range(PT - 1):
                        for ii in range(bi):
                            nc.tensor.transpose(
                                pst[p][:, ii, t, :],
                                rm[:, ii, 128 * p : 128 * (p + 1)],
                                ident[:, :],
                            )
                    for ii in range(bi):
                        nc.tensor.transpose(
                            pst[PT - 1][0:32, ii, t, :],
                            rm[:, ii, 512:MF],
                            ident[:, :],
                        )

                # ---- evac + reflect (ACT). For the last pair, the narrow
                # tile is evacuated+packed FIRST so the packed cascade can
                # run on DVE while the full W-tiles evacuate. ----
                def evac_narrow():
                    nc.scalar.copy(
                        tb4[0:32, :, PAD : PAD + H]
                        .rearrange("a b (t w) -> a b t w", t=NT),
                        pst[PT - 1][0:32, :, :, :],
                    )
                    nc.scalar.copy(tb4[0:32, :, 0:PAD], tb4[0:32, :, 2 * PAD : PAD : -1])
                    nc.scalar.copy(
                        tb4[0:32, :, H + PAD : HP], tb4[0:32, :, H + PAD - 2 : H - 2 : -1]
                    )
                    for ii in range(bi):
                        i = bi * b + ii
                        nc.scalar.copy(tbP4[32 * i : 32 * i + 14, :], tb4[0:14, ii, :])

                def evac_full(p):
                    nc.scalar.copy(
                        tbm[:, p, :, PAD : PAD + H]
                        .rearrange("a b (t w) -> a b t w", t=NT),
                        pst[p][:, :, :, :],
                    )
                    tbv = tbm[:, p]
                    nc.scalar.copy(tbv[:, :, 0:PAD], tbv[:, :, 2 * PAD : PAD : -1])
                    nc.scalar.copy(
                        tbv[:, :, H + PAD : HP], tbv[:, :, H + PAD - 2 : H - 2 : -1]
                    )

                if last:
                    evac_narrow()
                    for p in range(PT - 1):
                        evac_full(p)
                    # packed cascade fills the DVE stall during evacuation
                    g2 = cascp.tile([128, MF], BF16, tag="casc", name="g2")
                    g4 = cascp.tile([128, MF], BF16, tag="casc", name="g4")
                    g8 = cascp.tile([128, MF], BF16, tag="casc", name="g8")
                    nc.vector.tensor_tensor(
                        g2[:, 0:n2w], tbP4[:, 0:n2w], tbP4[:, 1 : n2w + 1], MIN
                    )
                    nc.vector.tensor_tensor(
                        g4[:, 0:n4w], g2[:, 0:n4w], g2[:, 2 : n4w + 2], MIN
                    )
                    nc.vector.tensor_tensor(
                        g8[:, 0:n8w], g4[:, 0:n8w], g4[:, 4 : n8w + 4], MIN
                    )
                    gdc = dcp.tile([128, HP], BF16, tag="gdc", name="gdc")
                    nc.vector.tensor_tensor(
                        gdc[:, :], g8[:, 0:HP], g8[:, PAD : HP + PAD], MIN
                    )
                    gdcs = dcsp.tile([128, HP], BF16, tag="gdcs", name="gdcs")
                    nc.scalar.activation(
                        gdcs[:, :], gdc[:, :], COPY,
                        accum_out=acc[:, n_acc - 1 : n_acc],
                    )
                else:
                    for p in range(PT - 1):
                        evac_full(p)
                    evac_narrow()

                # ---- pass 2 per full W-tile ----
                for p in range(PT - 1):
                    tbv = tbm[:, p]
                    h2 = cascp.tile([128, bi, MF], BF16, tag="hcasc", name=f"h2_{b}_{p}")
                    h4 = cascp.tile([128, bi, MF], BF16, tag="hcasc", name=f"h4_{b}_{p}")
                    h8 = cascp.tile([128, bi, MF], BF16, tag="hcasc", name=f"h8_{b}_{p}")
                    nc.vector.tensor_tensor(
                        h2[:, :, 0:n2w], tbv[:, :, 0:n2w], tbv[:, :, 1 : n2w + 1], MIN
                    )
                    nc.vector.tensor_tensor(
                        h4[:, :, 0:n4w], h2[:, :, 0:n4w], h2[:, :, 2 : n4w + 2], MIN
                    )
                    nc.vector.tensor_tensor(
                        h8[:, :, 0:n8w], h4[:, :, 0:n8w], h4[:, :, 4 : n8w + 4], MIN
                    )
                    dc = dcp.tile([128, bi, HP], BF16, tag="dc", name=f"dc_{b}_{p}")
                    nc.vector.tensor_tensor(
                        dc[:, :, :], h8[:, :, 0:HP], h8[:, :, PAD : HP + PAD], MIN
                    )
                    # per-tile sum on ScalarE (ACT accumulator)
                    k = b * (PT - 1) + p
                    dcs = dcsp.tile([128, bi, HP], BF16, tag="dcs", name=f"dcs_{b}_{p}")
                    nc.scalar.activation(
                        dcs[:, :, :], dc[:, :, :], COPY,
                        accum_out=acc[:, k : k + 1],
                    )

            nc.sync.dma_start(out[:, :], acc[:, :])

    return nc


_PROGRAM = None


def _get_program():
    global _PROGRAM
    if _PROGRAM is None:
        _PROGRAM = build_program()
        _PROGRAM.finalize()  # run Bacc passes (wait splitting, regalloc)
    return _PROGRAM


def kernel(generated_image):
    x = np.ascontiguousarray(np.asarray(generated_image), dtype=np.float32)
    assert x.shape == (B, C, H, W)
    nc = _get_program()
    shards = x.reshape(N_CORES, N_IMG, C, H, W)
    in_maps = [{"x": np.ascontiguousarray(shards[i])} for i in range(N_CORES)]
    res = run_bass_kernel_spmd(nc, in_maps, list(range(N_CORES)))
    total = float(np.sum([r["out"].astype(np.float64).sum() for r in res.results]))
    return np.array(-total / DEN, dtype=np.float32)
